# revision 41
# baseline (speedup 1.0000x reference)
"""Causal self-attention (B=4, T=2048, D=1024, H=16) on 8 trn2 NeuronCores.

Sharding: batch (4-way) x head-half (2-way tensor parallel) => 8 cores,
one uniform SPMD program (per-core differences are pure data: which batch's
x, which half of the QKV columns / proj columns each core receives).

Per core (batch b, head-half hh, 8 local heads), all matmul operands bf16
(fp32 PSUM accumulation):
  1. QKV: q^T/k^T computed in [qkv_col, token] layout (lhsT = W chunk,
     rhs = x^T chunk); v computed in [token, vcol] layout.  Emitted per
     512-token tile, interleaved with the attention of the query tile that
     just became computable, so the Tile scheduler overlaps ScalarE exp
     work with TensorE QKV/proj matmuls and the PE never idles long
     enough for the HAM clock gate to re-throttle.
  2. Attention per 512-wide query tile, streaming 128-wide key blocks
     (block-causal; fully-masked key blocks are skipped):
       S^T[k,q] = matmul(lhsT=k^T chunk, rhs=q^T tile)  for BOTH heads of
       a 128-partition group (row-packed in disjoint PE quadrants) into
       one 2-bank PSUM tile, then ONE ScalarE exp of width 1024 covers
       both heads (halves the per-instruction ACT overhead vs per-block
       exps).  Diagonal-region blocks compute the full query width and
       are zeroed after exp with a precomputed causal mask (DVE multiply).
       y_ext^T += matmul(lhsT=v_ext block, rhs=P^T): v_ext carries a ones
       column, so row HD of the accumulator is the softmax denominator l.
       Normalization: DVE reciprocal straight off the PSUM l row (bf16),
       rank-1 ones x r matmul broadcasts it across the head's 64
       partitions, one DVE multiply PSUM->SBUF per head.
  3. Pairwise AllGather of y^T (bf16, 512x512 per query tile) between the
     two cores sharing a batch => full y^T [1024, 512] on both.
  4. proj: out[:, 512 cols of this core] = y @ W_p[:, cols] (+bias),
     column-sharded => the host only concatenates, no reduction anywhere.
"""

import os
import sys
from dataclasses import dataclass

import ml_dtypes
import numpy as np

sys.path.insert(0, "/opt/trn_rl_repo")

import concourse.mybir as mybir  # noqa: E402
import concourse.tile as tile  # noqa: E402
from concourse import bacc  # noqa: E402
from concourse.bass import ds, ts  # noqa: E402

P = 128
F32 = mybir.dt.float32
BF16 = mybir.dt.bfloat16
AF = mybir.ActivationFunctionType
ALU = mybir.AluOpType
BF16NP = ml_dtypes.bfloat16


@dataclass(frozen=True)
class Cfg:
    T: int = 2048          # sequence length
    D: int = 1024          # model dim (QKV contraction dim)
    H_LOC: int = 8         # heads per core
    HD: int = 64           # head dim
    TT: int = 512          # token tile width in the QKV phase
    QT: int = 512          # query tile width in the attention phase
    n_groups: int = 2      # cores sharing a batch (pairwise AllGather)
    scale: float = 64 ** -0.5

    @property
    def DH(self):          # local head dims (y^T rows contributed per core)
        return self.H_LOC * self.HD

    @property
    def GDH(self):         # proj contraction dim (= model dim)
        return self.n_groups * self.DH

    @property
    def DCH(self):
        return self.D // P

    @property
    def NHP(self):         # 128-partition groups of local head dims
        return self.DH // P

    @property
    def HPG(self):         # heads per 128-partition group
        return P // self.HD

    @property
    def NTT(self):
        return self.T // self.TT

    @property
    def NQT(self):
        return self.T // self.QT

    @property
    def CB(self):          # 128-wide column blocks of the local q/k cols
        return self.DH // P


FULL = Cfg()


def build_nc(c: Cfg, n_cores: int = 8, with_bias: bool = True):
    """Build the (uniform SPMD) Bass program for one core."""
    assert c.T % c.TT == 0 and c.T % c.QT == 0 and c.QT % P == 0
    assert c.D % P == 0 and c.DH % P == 0 and c.TT % P == 0
    use_cc = c.n_groups > 1

    nc = bacc.Bacc(
        "TRN2", target_bir_lowering=False, debug=False, num_devices=n_cores
    )
    xT = nc.dram_tensor("xT", [c.D, c.T], BF16, kind="ExternalInput").ap()
    wq = nc.dram_tensor("wq", [c.D, c.DH], BF16, kind="ExternalInput").ap()
    wk = nc.dram_tensor("wk", [c.D, c.DH], BF16, kind="ExternalInput").ap()
    wv = nc.dram_tensor("wv", [c.D, c.DH], BF16, kind="ExternalInput").ap()
    bq = nc.dram_tensor("bq", [c.DH], F32, kind="ExternalInput").ap()
    bk = nc.dram_tensor("bk", [c.DH], F32, kind="ExternalInput").ap()
    bv = nc.dram_tensor("bv", [1, c.DH], BF16, kind="ExternalInput").ap()
    wp = nc.dram_tensor("wp", [c.GDH, c.DH], BF16, kind="ExternalInput").ap()
    bp = nc.dram_tensor("bp", [1, c.DH], BF16, kind="ExternalInput").ap()
    oc = max(P, (c.T // P) * c.H_LOC)
    onesin = nc.dram_tensor("onesin", [P, oc], BF16, kind="ExternalInput").ap()
    esel = nc.dram_tensor("esel", [c.H_LOC, c.NHP * P], BF16,
                          kind="ExternalInput").ap()
    out = nc.dram_tensor("out", [c.T, c.DH], F32, kind="ExternalOutput").ap()

    groups = [[g * c.n_groups + i for i in range(c.n_groups)]
              for g in range(max(1, n_cores // c.n_groups))]
    ndiag = c.QT // P

    with tile.TileContext(nc) as tc:
        with (
            tc.tile_pool(name="const", bufs=1) as cst,
            tc.tile_pool(name="kv", bufs=1) as kv,
            tc.tile_pool(name="wts", bufs=1) as wts,
            tc.tile_pool(name="xt", bufs=2) as xtp,
            tc.tile_pool(name="pt", bufs=8) as ptp,
            tc.tile_pool(name="yt", bufs=2) as ytp,
            tc.tile_pool(name="yu", bufs=10) as yup,
            tc.tile_pool(name="lr", bufs=2) as lrp,
            tc.tile_pool(name="yag", bufs=2) as yagp,
            tc.tile_pool(name="osb", bufs=2) as osbp,
            tc.tile_pool(name="ps_s", bufs=3, space="PSUM") as ps_s,
            tc.tile_pool(name="ps_y", bufs=2, space="PSUM") as ps_y,
            tc.tile_pool(name="dram", bufs=2, space="DRAM") as drp,
        ):
            # ---- x tile 0 + weights first: these gate the first matmul
            # chain, so their DMAs are issued before everything else
            # (strided rearrange loads fan out across many DMA engines)
            xT_r = xT.rearrange("(ch p) t -> p ch t", p=P)
            xt0 = xtp.tile([P, c.DCH, c.TT], BF16, tag="xt", name="xt")
            wq_sb = wts.tile([P, c.DCH, c.DH], BF16)
            wk_sb = wts.tile([P, c.DCH, c.DH], BF16)
            wv_sb = wts.tile([P, c.DCH, c.DH], BF16)
            wp_sb = wts.tile([P, c.GDH // P, c.DH], BF16)
            nc.gpsimd.dma_start(
                wk_sb, wk.rearrange("(ch p) n -> p ch n", p=P))
            nc.sync.dma_start(
                wq_sb, wq.rearrange("(ch p) n -> p ch n", p=P))
            nc.sync.dma_start(xt0, xT_r[:, :, ts(0, c.TT)])
            nc.gpsimd.dma_start(
                wv_sb, wv.rearrange("(ch p) n -> p ch n", p=P))
            nc.gpsimd.dma_start(
                wp_sb, wp.rearrange("(ch p) n -> p ch n", p=P))

            # ---- constants ----
            ones_row = cst.tile([1, P], BF16)
            nc.sync.dma_start(ones_row, onesin[0:1, 0:P])
            bq_sb = cst.tile([P, c.CB], F32)
            nc.sync.dma_start(bq_sb, bq.rearrange("(cb p) -> p cb", p=P))
            bk_sb = cst.tile([P, c.CB], F32)
            nc.sync.dma_start(bk_sb, bk.rearrange("(cb p) -> p cb", p=P))
            bv_row = cst.tile([1, c.DH], BF16)
            nc.sync.dma_start(bv_row, bv)
            bp_row = cst.tile([1, c.DH], BF16)
            nc.sync.dma_start(bp_row, bp)
            esel_sb = cst.tile([c.H_LOC, c.NHP * P], BF16)
            nc.sync.dma_start(esel_sb, esel)
            # causal triangle mask for the 128-wide diagonal strip:
            # mask[k, j] keeps where j - k >= 0 (j = query, k = key)
            mask_tri = cst.tile([P, P], BF16)
            nc.vector.memset(mask_tri, 1.0)
            nc.gpsimd.affine_select(
                mask_tri, mask_tri,
                compare_op=ALU.is_ge, fill=0.0, base=0,
                pattern=[[1, P]], channel_multiplier=-1,
            )

            # ---- persistent K^T / Q^T / V(+ones) and resident weights ----
            kT = kv.tile([P, c.NHP, c.T], BF16)
            qT = kv.tile([P, c.NHP, c.T], BF16)
            v = kv.tile([P, c.T // P, c.H_LOC, c.HD + 1], BF16)
            nc.vector.memset(v[:, :, :, c.HD:c.HD + 1], 1.0)

            xts = {0: xt0}

            def prefetch_xt(tt):
                if tt < c.NTT and tt not in xts:
                    xt = xtp.tile([P, c.DCH, c.TT], BF16, tag="xt",
                                  name="xt")
                    nc.sync.dma_start(xt, xT_r[:, :, ts(tt, c.TT)])
                    xts[tt] = xt

            def kq_chain(tt, dst, w_sb, b_sb, cb):
                pst = ps_s.tile([P, max(c.TT, c.DH)], F32,
                                 tag="pss", name="pst")[:, :c.TT]
                for dc in range(c.DCH):
                    nc.tensor.matmul(
                        pst,
                        w_sb[:, dc, ts(cb, P)],
                        xts[tt][:, dc, :],
                        start=(dc == 0),
                        stop=(dc == c.DCH - 1),
                    )
                nc.vector.tensor_tensor(
                    dst[:, cb, ts(tt, c.TT)], pst,
                    b_sb[:, cb:cb + 1].to_broadcast((P, c.TT)),
                    ALU.add,
                )

            def v_chain(tt, tb):
                gtb = tt * (c.TT // P) + tb
                psv = ps_s.tile([P, max(c.TT, c.DH)], F32,
                                 tag="pss", name="psv")[:, :c.DH]
                for dc in range(c.DCH):
                    nc.tensor.matmul(
                        psv,
                        xts[tt][:, dc, ts(tb, P)],
                        wv_sb[:, dc, :],
                        start=(dc == 0),
                        stop=(not with_bias and dc == c.DCH - 1),
                    )
                if with_bias:
                    nc.tensor.matmul(
                        psv, ones_row[0:1, 0:P], bv_row,
                        start=False, stop=True,
                    )
                nc.vector.tensor_copy(
                    v[:, gtb, :, 0:c.HD],
                    psv.rearrange("p (h d) -> p h d", d=c.HD),
                )

            def emit_kq(tt, cb):
                kq_chain(tt, kT, wk_sb, bk_sb, cb)
                kq_chain(tt, qT, wq_sb, bq_sb, cb)

            def emit_v(tt):
                for tb in range(c.TT // P):
                    v_chain(tt, tb)

            def att_s_exp(q0, qw, hps):
                """S^T + exp for queries [q0, q0+qw), with one-group S^T
                lookahead so the ScalarE exp stream never waits on PE
                scheduling.  Returns the exp'd P^T tiles per group for a
                later att_pv."""
                nkb = (q0 + qw) // P
                sexps = {}
                for hp in hps:
                    def s_pair(kb):
                        off = max(0, kb * P - q0)
                        pss = ps_s.tile([P, c.HPG, c.QT], F32, tag="pss",
                                        name="pss")[:, :, :qw]
                        for hs in range(c.HPG):
                            pb = hs * c.HD
                            nc.tensor.matmul(
                                pss[:, hs, off:],
                                kT[pb:pb + c.HD, hp, ts(kb, P)],
                                qT[pb:pb + c.HD, hp, ds(q0 + off,
                                                        qw - off)],
                                start=True, stop=True,
                            )
                        return pss, off

                    cur = s_pair(0)
                    pts = []
                    for kb in range(nkb):
                        nxt = s_pair(kb + 1) if kb + 1 < nkb else None
                        pss, off = cur
                        pt = ptp.tile([P, c.HPG, c.QT], BF16, tag="pt",
                                      name="pt")[:, :, :qw]
                        nc.scalar.activation(
                            pt[:, :, off:], pss[:, :, off:],
                            AF.Exp, scale=c.scale)
                        if kb * P >= q0:
                            # triangle mask on the 128 cols at the diagonal
                            for hs in range(c.HPG):
                                nc.vector.tensor_tensor(
                                    pt[:, hs, off:off + P],
                                    pt[:, hs, off:off + P],
                                    mask_tri, ALU.mult,
                                )
                        pts.append((pt, off))
                        cur = nxt
                    sexps[hp] = pts
                return sexps

            def att_pv(q0, qw, sexps, l_all, yus):
                """PV accumulation over the exp'd P^T tiles, then stage
                y_unnorm^T + denominator rows out of PSUM fast (frees the
                psys banks for the next partition group); the softmax
                division happens later off the critical path: one batched
                reciprocal per pass."""
                nkb = (q0 + qw) // P
                for hp, pts in sexps.items():
                    psys = [ps_y.tile([c.HD + 1, c.QT], F32, tag="psy",
                                      name=f"psy{hs}")[:, :qw]
                            for hs in range(c.HPG)]
                    for kb, (pt, off) in enumerate(pts):
                        for hs in range(c.HPG):
                            nc.tensor.matmul(
                                psys[hs][:, off:],
                                v[:, kb, hp * c.HPG + hs, :],
                                pt[:, hs, off:],
                                start=(kb == 0),
                                stop=(kb == nkb - 1),
                            )
                    for hs in range(c.HPG):
                        h = hp * c.HPG + hs
                        yu = yup.tile([c.HD + 1, c.QT], F32, tag="yu",
                                      name=f"yu{hs}")[:, :qw]
                        nc.vector.tensor_copy(yu, psys[hs])
                        nc.gpsimd.dma_start(
                            l_all[h:h + 1, 0:qw], yu[c.HD:c.HD + 1, :])
                        yus[h] = yu

            def att_pass(q0, qw, hps, l_all, yus):
                """Fused S/exp/PV flow (one-group S^T lookahead) for the
                steady-state passes, whose QKV inputs are fully resident
                by the time they run."""
                nkb = (q0 + qw) // P
                for hp in hps:
                    psys = [ps_y.tile([c.HD + 1, c.QT], F32, tag="psy",
                                      name=f"psy{hs}")[:, :qw]
                            for hs in range(c.HPG)]

                    def s_pair(kb):
                        off = max(0, kb * P - q0)
                        pss = ps_s.tile([P, c.HPG, c.QT], F32, tag="pss",
                                        name="pss")[:, :, :qw]
                        for hs in range(c.HPG):
                            pb = hs * c.HD
                            nc.tensor.matmul(
                                pss[:, hs, off:],
                                kT[pb:pb + c.HD, hp, ts(kb, P)],
                                qT[pb:pb + c.HD, hp, ds(q0 + off,
                                                        qw - off)],
                                start=True, stop=True,
                            )
                        return pss, off

                    cur = s_pair(0)
                    for kb in range(nkb):
                        nxt = s_pair(kb + 1) if kb + 1 < nkb else None
                        pss, off = cur
                        pt = ptp.tile([P, c.HPG, c.QT], BF16, tag="pt",
                                      name="pt")[:, :, :qw]
                        nc.scalar.activation(
                            pt[:, :, off:], pss[:, :, off:],
                            AF.Exp, scale=c.scale)
                        if kb * P >= q0:
                            for hs in range(c.HPG):
                                nc.vector.tensor_tensor(
                                    pt[:, hs, off:off + P],
                                    pt[:, hs, off:off + P],
                                    mask_tri, ALU.mult,
                                )
                        for hs in range(c.HPG):
                            nc.tensor.matmul(
                                psys[hs][:, off:],
                                v[:, kb, hp * c.HPG + hs, :],
                                pt[:, hs, off:],
                                start=(kb == 0),
                                stop=(kb == nkb - 1),
                            )
                        cur = nxt
                    for hs in range(c.HPG):
                        h = hp * c.HPG + hs
                        yu = yup.tile([c.HD + 1, c.QT], F32, tag="yu",
                                      name=f"yu{hs}")[:, :qw]
                        nc.vector.tensor_copy(yu, psys[hs])
                        nc.gpsimd.dma_start(
                            l_all[h:h + 1, 0:qw], yu[c.HD:c.HD + 1, :])
                        yus[h] = yu

            def norm_pass(qw, yus, l_all):
                r_all = lrp.tile([c.H_LOC, c.QT], BF16, tag="rall",
                                 name="r_all")
                with nc.allow_low_precision(
                    reason="1/l rounded to bf16 for the broadcast matmul"
                ):
                    nc.vector.reciprocal(r_all[:, 0:qw], l_all[:, 0:qw])
                yt_q = ytp.tile([P, c.NHP, c.QT], BF16,
                                tag="yt_q", name="yt_q")[:, :, :qw]
                for hp in range(c.NHP):
                    psr = ps_s.tile([P, max(c.TT, c.DH)], F32,
                                     tag="pss", name="psr")[:, :qw]
                    nc.tensor.matmul(
                        psr, esel_sb[:, ts(hp, P)], r_all[:, 0:qw],
                        start=True, stop=True,
                    )
                    for hs in range(c.HPG):
                        pb = hs * c.HD
                        nc.vector.tensor_tensor(
                            yt_q[pb:pb + c.HD, hp, :],
                            yus[hp * c.HPG + hs][0:c.HD, :],
                            psr[pb:pb + c.HD, :],
                            ALU.mult,
                        )
                return yt_q

            def ship_ag(qw, yt_q):
                # ship y^T to DRAM + pairwise AllGather along the dims
                # axis (runs on the CC cores, overlapped with compute)
                y_loc = drp.tile([c.DH, qw], BF16,
                                 tag=f"yloc{qw}", name="y_loc")
                nc.sync.dma_start(
                    y_loc.rearrange("(hp p) t -> p hp t", p=P), yt_q)
                if use_cc:
                    y_ag = drp.tile([c.GDH, qw], BF16,
                                    tag=f"ygat{qw}", name="y_ag")
                    nc.gpsimd.collective_compute(
                        "AllGather", ALU.bypass,
                        replica_groups=groups,
                        ins=[y_loc.opt()], outs=[y_ag.opt()],
                    )
                else:
                    y_ag = y_loc
                return y_ag

            def proj_pass(q0, qw, y_ag):
                # column-sharded proj on the gathered y^T; emitted one
                # pass later than its AllGather so a late AllGather can
                # never block the PE queue (the scheduler's cost model
                # underestimates collective latency).
                yag_sb = yagp.tile([P, c.GDH // P, c.QT], BF16,
                                   name="yag_sb")[:, :, :qw]
                nc.sync.dma_start(
                    yag_sb, y_ag.rearrange("(ch p) t -> p ch t", p=P))
                for tb in range(qw // P):
                    pso = ps_s.tile([P, max(c.TT, c.DH)], F32,
                                     tag="pss", name="pso")[:, :c.DH]
                    for c2 in range(c.GDH // P):
                        nc.tensor.matmul(
                            pso,
                            yag_sb[:, c2, ts(tb, P)],
                            wp_sb[:, c2, :],
                            start=(c2 == 0),
                            stop=(not with_bias
                                  and c2 == c.GDH // P - 1),
                        )
                    if with_bias:
                        nc.tensor.matmul(
                            pso, ones_row[0:1, 0:P], bp_row,
                            start=False, stop=True,
                        )
                    osb = osbp.tile([P, c.DH], F32)
                    nc.vector.tensor_copy(osb, pso)
                    nc.sync.dma_start(
                        out[ds(q0 + tb * P, P), :], osb)

            # ===== the pipeline: the last tile is split in two query
            # halves so its AllGather+proj tail is halved; each tile's
            # QKV is emitted between the next attention pass and its
            # normalization so the PE has unblocked matmuls while the
            # AllGather flies and the reciprocal runs.
            passes = []
            for tt in range(c.NTT):
                if tt == c.NTT - 1 and c.QT >= 2 * P:
                    hqw = c.QT // 2
                    passes.append((tt, tt * c.QT, hqw, True))
                    passes.append((tt, tt * c.QT + hqw, hqw, False))
                else:
                    passes.append((tt, tt * c.QT, c.QT, True))

            pending_proj = None
            for tt, q0, qw, first in passes:
                if first:
                    prefetch_xt(tt + 1)
                l_all = lrp.tile([c.H_LOC, c.QT], F32, tag="lall",
                                 name="l_all")
                yus = {}
                if tt == 0 and first:
                    # pipeline head: each K/Q chain pair immediately
                    # unblocks its partition group's S^T + exp stream, so
                    # the ScalarE exp stream starts after just two chains.
                    # Group 0's PVs are deferred past the V chains (which
                    # the PE fills in while group 0's exps run).
                    for hp in range(c.NHP):
                        emit_kq(0, hp)
                        se = att_s_exp(q0, qw, [hp])
                        if hp == 0:
                            emit_v(0)
                        att_pv(q0, qw, se, l_all, yus)
                else:
                    att_pass(q0, qw, list(range(c.NHP)), l_all, yus)
                if first and tt + 1 < c.NTT:
                    for cb in range(c.CB):
                        emit_kq(tt + 1, cb)
                    emit_v(tt + 1)
                yt_q = norm_pass(qw, yus, l_all)
                y_ag = ship_ag(qw, yt_q)
                if pending_proj is not None:
                    proj_pass(*pending_proj)
                pending_proj = (q0, qw, y_ag)
            proj_pass(*pending_proj)

    nc.compile()
    return nc


def shard_inputs(c: Cfg, x, w_qkv, b_qkv, w_proj, b_proj, n_cores=8):
    """Full fp32 inputs -> per-core input maps (host-side marshalling).

    Matmul operands are cast to bf16 on the host; q/k biases stay fp32
    (applied on the f32 PSUM before the bf16 round)."""
    D, DH = c.D, c.DH
    oc = max(128, (c.T // 128) * c.H_LOC)
    ones = np.ones((128, oc), BF16NP)
    esel = np.zeros((c.H_LOC, c.NHP * 128), BF16NP)
    for h in range(c.H_LOC):
        hp, sub = h // c.HPG, h % c.HPG
        esel[h, hp * 128 + sub * c.HD: hp * 128 + (sub + 1) * c.HD] = 1

    maps = []
    for core in range(n_cores):
        b, hh = core // c.n_groups, core % c.n_groups
        sl = slice(hh * DH, (hh + 1) * DH)
        maps.append({
            "xT": np.ascontiguousarray(x[b].T).astype(BF16NP),
            "wq": np.ascontiguousarray(
                w_qkv[:, 0 * D:1 * D][:, sl]).astype(BF16NP),
            "wk": np.ascontiguousarray(
                w_qkv[:, 1 * D:2 * D][:, sl]).astype(BF16NP),
            "wv": np.ascontiguousarray(
                w_qkv[:, 2 * D:3 * D][:, sl]).astype(BF16NP),
            "bq": np.ascontiguousarray(
                b_qkv[0 * D:1 * D][sl], dtype=np.float32),
            "bk": np.ascontiguousarray(
                b_qkv[1 * D:2 * D][sl], dtype=np.float32),
            "bv": np.ascontiguousarray(
                b_qkv[2 * D:3 * D][sl]).reshape(1, DH).astype(BF16NP),
            "wp": np.ascontiguousarray(w_proj[:, sl]).astype(BF16NP),
            "bp": np.ascontiguousarray(
                b_proj[sl]).reshape(1, DH).astype(BF16NP),
            "onesin": ones,
            "esel": esel,
        })
    return maps


def gather_outputs(c: Cfg, results, n_cores=8):
    B = n_cores // c.n_groups
    out = np.empty((B, c.T, c.GDH), dtype=np.float32)
    for core in range(n_cores):
        b, hh = core // c.n_groups, core % c.n_groups
        out[b][:, hh * c.DH:(hh + 1) * c.DH] = results[core]["out"]
    return out


_NC_CACHE: dict = {}


def kernel(**inputs) -> np.ndarray:
    from concourse.bass_utils import run_bass_kernel_spmd

    c = FULL
    n_cores = 8
    wb = bool(np.any(inputs["b_qkv"]) or np.any(inputs["b_proj"]))
    key = (c, n_cores, wb)
    if key not in _NC_CACHE:
        _NC_CACHE[key] = build_nc(c, n_cores, with_bias=wb)
    nc = _NC_CACHE[key]
    in_maps = shard_inputs(
        c, inputs["x"], inputs["w_qkv"], inputs["b_qkv"],
        inputs["w_proj"], inputs["b_proj"], n_cores,
    )
    res = run_bass_kernel_spmd(
        nc, in_maps, core_ids=list(range(n_cores)),
        trace=bool(int(os.environ.get("KERNEL_TRACE", "0"))),
    )
    kernel.last_results = res
    return gather_outputs(c, res.results, n_cores)


# revision 42
# speedup vs baseline: 1.0652x; 1.0652x over previous
"""Causal self-attention (B=4, T=2048, D=1024, H=16) on 8 trn2 NeuronCores.

Sharding: batch (4-way) x head-half (2-way tensor parallel) => 8 cores,
one uniform SPMD program (per-core differences are pure data: which batch's
x, which half of the QKV columns / proj columns each core receives).

Per core (batch b, head-half hh, 8 local heads), all matmul operands bf16
(fp32 PSUM accumulation):
  1. QKV: q^T/k^T computed in [qkv_col, token] layout (lhsT = W chunk,
     rhs = x^T chunk); v computed in [token, vcol] layout.  Emitted per
     512-token tile, interleaved with the attention of the query tile that
     just became computable, so the Tile scheduler overlaps ScalarE exp
     work with TensorE QKV/proj matmuls and the PE never idles long
     enough for the HAM clock gate to re-throttle.
  2. Attention per 512-wide query tile, streaming 128-wide key blocks
     (block-causal; fully-masked key blocks are skipped):
       S^T[k,q] = matmul(lhsT=k^T chunk, rhs=q^T tile)  for BOTH heads of
       a 128-partition group (row-packed in disjoint PE quadrants) into
       one 2-bank PSUM tile, then ONE ScalarE exp of width 1024 covers
       both heads (halves the per-instruction ACT overhead vs per-block
       exps).  Diagonal-region blocks compute the full query width and
       are zeroed after exp with a precomputed causal mask (DVE multiply).
       y_ext^T += matmul(lhsT=v_ext block, rhs=P^T): v_ext carries a ones
       column, so row HD of the accumulator is the softmax denominator l.
       Normalization: DVE reciprocal straight off the PSUM l row (bf16),
       rank-1 ones x r matmul broadcasts it across the head's 64
       partitions, one DVE multiply PSUM->SBUF per head.
  3. Pairwise AllGather of y^T (bf16, 512x512 per query tile) between the
     two cores sharing a batch => full y^T [1024, 512] on both.
  4. proj: out[:, 512 cols of this core] = y @ W_p[:, cols] (+bias),
     column-sharded => the host only concatenates, no reduction anywhere.
"""

import os
import sys
from dataclasses import dataclass

import ml_dtypes
import numpy as np

sys.path.insert(0, "/opt/trn_rl_repo")

import concourse.mybir as mybir  # noqa: E402
import concourse.tile as tile  # noqa: E402
from concourse import bacc  # noqa: E402
from concourse.bass import ds, ts  # noqa: E402

P = 128
F32 = mybir.dt.float32
BF16 = mybir.dt.bfloat16
AF = mybir.ActivationFunctionType
ALU = mybir.AluOpType
BF16NP = ml_dtypes.bfloat16


@dataclass(frozen=True)
class Cfg:
    T: int = 2048          # sequence length
    D: int = 1024          # model dim (QKV contraction dim)
    H_LOC: int = 8         # heads per core
    HD: int = 64           # head dim
    TT: int = 512          # token tile width in the QKV phase
    QT: int = 512          # query tile width in the attention phase
    n_groups: int = 2      # cores sharing a batch (pairwise AllGather)
    scale: float = 64 ** -0.5

    @property
    def DH(self):          # local head dims (y^T rows contributed per core)
        return self.H_LOC * self.HD

    @property
    def GDH(self):         # proj contraction dim (= model dim)
        return self.n_groups * self.DH

    @property
    def DCH(self):
        return self.D // P

    @property
    def NHP(self):         # 128-partition groups of local head dims
        return self.DH // P

    @property
    def HPG(self):         # heads per 128-partition group
        return P // self.HD

    @property
    def NTT(self):
        return self.T // self.TT

    @property
    def NQT(self):
        return self.T // self.QT

    @property
    def CB(self):          # 128-wide column blocks of the local q/k cols
        return self.DH // P


FULL = Cfg()


def build_nc(c: Cfg, n_cores: int = 8, with_bias: bool = True):
    """Build the (uniform SPMD) Bass program for one core."""
    assert c.T % c.TT == 0 and c.T % c.QT == 0 and c.QT % P == 0
    assert c.D % P == 0 and c.DH % P == 0 and c.TT % P == 0
    use_cc = c.n_groups > 1

    nc = bacc.Bacc(
        "TRN2", target_bir_lowering=False, debug=False, num_devices=n_cores
    )
    xT = nc.dram_tensor("xT", [c.D, c.T], BF16, kind="ExternalInput").ap()
    wq = nc.dram_tensor("wq", [c.D, c.DH], BF16, kind="ExternalInput").ap()
    wk = nc.dram_tensor("wk", [c.D, c.DH], BF16, kind="ExternalInput").ap()
    wv = nc.dram_tensor("wv", [c.D, c.DH], BF16, kind="ExternalInput").ap()
    bq = nc.dram_tensor("bq", [c.DH], F32, kind="ExternalInput").ap()
    bk = nc.dram_tensor("bk", [c.DH], F32, kind="ExternalInput").ap()
    bv = nc.dram_tensor("bv", [1, c.DH], BF16, kind="ExternalInput").ap()
    wp = nc.dram_tensor("wp", [c.GDH, c.DH], BF16, kind="ExternalInput").ap()
    bp = nc.dram_tensor("bp", [1, c.DH], BF16, kind="ExternalInput").ap()
    oc = max(P, (c.T // P) * c.H_LOC)
    onesin = nc.dram_tensor("onesin", [P, oc], BF16, kind="ExternalInput").ap()
    esel = nc.dram_tensor("esel", [c.H_LOC, c.NHP * P], BF16,
                          kind="ExternalInput").ap()
    out = nc.dram_tensor("out", [c.T, c.DH], F32, kind="ExternalOutput").ap()

    groups = [[g * c.n_groups + i for i in range(c.n_groups)]
              for g in range(max(1, n_cores // c.n_groups))]
    ndiag = c.QT // P

    with tile.TileContext(nc) as tc:
        with (
            tc.tile_pool(name="const", bufs=1) as cst,
            tc.tile_pool(name="kv", bufs=1) as kv,
            tc.tile_pool(name="wts", bufs=1) as wts,
            tc.tile_pool(name="xt", bufs=2) as xtp,
            tc.tile_pool(name="pt", bufs=8) as ptp,
            tc.tile_pool(name="yt", bufs=2) as ytp,
            tc.tile_pool(name="yu", bufs=10) as yup,
            tc.tile_pool(name="lr", bufs=2) as lrp,
            tc.tile_pool(name="yag", bufs=2) as yagp,
            tc.tile_pool(name="osb", bufs=2) as osbp,
            tc.tile_pool(name="ps_mm", bufs=2, space="PSUM") as ps_mm,
            tc.tile_pool(name="ps_s", bufs=2, space="PSUM") as ps_s,
            tc.tile_pool(name="ps_y", bufs=2, space="PSUM") as ps_y,
            tc.tile_pool(name="dram", bufs=2, space="DRAM") as drp,
        ):
            # ---- x tile 0 + weights first: these gate the first matmul
            # chain, so their DMAs are issued before everything else
            # (strided rearrange loads fan out across many DMA engines)
            xT_r = xT.rearrange("(ch p) t -> p ch t", p=P)
            xt0 = xtp.tile([P, c.DCH, c.TT], BF16, tag="xt", name="xt")
            wq_sb = wts.tile([P, c.DCH, c.DH], BF16)
            wk_sb = wts.tile([P, c.DCH, c.DH], BF16)
            wv_sb = wts.tile([P, c.DCH, c.DH], BF16)
            wp_sb = wts.tile([P, c.GDH // P, c.DH], BF16)
            nc.gpsimd.dma_start(
                wk_sb, wk.rearrange("(ch p) n -> p ch n", p=P))
            nc.sync.dma_start(
                wq_sb, wq.rearrange("(ch p) n -> p ch n", p=P))
            nc.sync.dma_start(xt0, xT_r[:, :, ts(0, c.TT)])
            nc.gpsimd.dma_start(
                wv_sb, wv.rearrange("(ch p) n -> p ch n", p=P))
            nc.gpsimd.dma_start(
                wp_sb, wp.rearrange("(ch p) n -> p ch n", p=P))

            # ---- constants ----
            ones_row = cst.tile([1, P], BF16)
            nc.sync.dma_start(ones_row, onesin[0:1, 0:P])
            bq_sb = cst.tile([P, c.CB], F32)
            nc.sync.dma_start(bq_sb, bq.rearrange("(cb p) -> p cb", p=P))
            bk_sb = cst.tile([P, c.CB], F32)
            nc.sync.dma_start(bk_sb, bk.rearrange("(cb p) -> p cb", p=P))
            bv_row = cst.tile([1, c.DH], BF16)
            nc.sync.dma_start(bv_row, bv)
            bp_row = cst.tile([1, c.DH], BF16)
            nc.sync.dma_start(bp_row, bp)
            esel_sb = cst.tile([c.H_LOC, c.NHP * P], BF16)
            nc.sync.dma_start(esel_sb, esel)
            # causal triangle mask for the 128-wide diagonal strip:
            # mask[k, j] keeps where j - k >= 0 (j = query, k = key)
            mask_tri = cst.tile([P, P], BF16)
            nc.vector.memset(mask_tri, 1.0)
            nc.gpsimd.affine_select(
                mask_tri, mask_tri,
                compare_op=ALU.is_ge, fill=0.0, base=0,
                pattern=[[1, P]], channel_multiplier=-1,
            )

            # ---- persistent K^T / Q^T / V(+ones) and resident weights ----
            kT = kv.tile([P, c.NHP, c.T], BF16)
            qT = kv.tile([P, c.NHP, c.T], BF16)
            v = kv.tile([P, c.T // P, c.H_LOC, c.HD + 1], BF16)
            nc.vector.memset(v[:, :, :, c.HD:c.HD + 1], 1.0)

            xts = {0: xt0}

            def prefetch_xt(tt):
                if tt < c.NTT and tt not in xts:
                    xt = xtp.tile([P, c.DCH, c.TT], BF16, tag="xt",
                                  name="xt")
                    nc.sync.dma_start(xt, xT_r[:, :, ts(tt, c.TT)])
                    xts[tt] = xt

            def kq_chain(tt, dst, w_sb, b_sb, cb):
                pst = ps_mm.tile([P, max(c.TT, c.DH)], F32,
                                 tag="mm", name="pst")[:, :c.TT]
                for dc in range(c.DCH):
                    nc.tensor.matmul(
                        pst,
                        w_sb[:, dc, ts(cb, P)],
                        xts[tt][:, dc, :],
                        start=(dc == 0),
                        stop=(dc == c.DCH - 1),
                    )
                nc.vector.tensor_tensor(
                    dst[:, cb, ts(tt, c.TT)], pst,
                    b_sb[:, cb:cb + 1].to_broadcast((P, c.TT)),
                    ALU.add,
                )

            def v_chain(tt, tb):
                gtb = tt * (c.TT // P) + tb
                psv = ps_mm.tile([P, max(c.TT, c.DH)], F32,
                                 tag="mm", name="psv")[:, :c.DH]
                for dc in range(c.DCH):
                    nc.tensor.matmul(
                        psv,
                        xts[tt][:, dc, ts(tb, P)],
                        wv_sb[:, dc, :],
                        start=(dc == 0),
                        stop=(not with_bias and dc == c.DCH - 1),
                    )
                if with_bias:
                    nc.tensor.matmul(
                        psv, ones_row[0:1, 0:P], bv_row,
                        start=False, stop=True,
                    )
                nc.vector.tensor_copy(
                    v[:, gtb, :, 0:c.HD],
                    psv.rearrange("p (h d) -> p h d", d=c.HD),
                )

            def emit_kq(tt, cb):
                kq_chain(tt, kT, wk_sb, bk_sb, cb)
                kq_chain(tt, qT, wq_sb, bq_sb, cb)

            def emit_v(tt):
                for tb in range(c.TT // P):
                    v_chain(tt, tb)

            def att_s_exp(q0, qw, hps):
                """S^T + exp for queries [q0, q0+qw), with one-group S^T
                lookahead so the ScalarE exp stream never waits on PE
                scheduling.  Returns the exp'd P^T tiles per group for a
                later att_pv."""
                nkb = (q0 + qw) // P
                sexps = {}
                for hp in hps:
                    def s_pair(kb):
                        off = max(0, kb * P - q0)
                        pss = ps_s.tile([P, c.HPG, c.QT], F32, tag="pss",
                                        name="pss")[:, :, :qw]
                        for hs in range(c.HPG):
                            pb = hs * c.HD
                            nc.tensor.matmul(
                                pss[:, hs, off:],
                                kT[pb:pb + c.HD, hp, ts(kb, P)],
                                qT[pb:pb + c.HD, hp, ds(q0 + off,
                                                        qw - off)],
                                start=True, stop=True,
                            )
                        return pss, off

                    cur = s_pair(0)
                    pts = []
                    for kb in range(nkb):
                        nxt = s_pair(kb + 1) if kb + 1 < nkb else None
                        pss, off = cur
                        pt = ptp.tile([P, c.HPG, c.QT], BF16, tag="pt",
                                      name="pt")[:, :, :qw]
                        nc.scalar.activation(
                            pt[:, :, off:], pss[:, :, off:],
                            AF.Exp, scale=c.scale)
                        if kb * P >= q0:
                            # triangle mask on the 128 cols at the diagonal
                            for hs in range(c.HPG):
                                nc.vector.tensor_tensor(
                                    pt[:, hs, off:off + P],
                                    pt[:, hs, off:off + P],
                                    mask_tri, ALU.mult,
                                )
                        pts.append((pt, off))
                        cur = nxt
                    sexps[hp] = pts
                return sexps

            def att_pv(q0, qw, sexps, l_all, yus):
                """PV accumulation over the exp'd P^T tiles, then stage
                y_unnorm^T + denominator rows out of PSUM fast (frees the
                psys banks for the next partition group); the softmax
                division happens later off the critical path: one batched
                reciprocal per pass."""
                nkb = (q0 + qw) // P
                for hp, pts in sexps.items():
                    psys = [ps_y.tile([c.HD + 1, c.QT], F32, tag="psy",
                                      name=f"psy{hs}")[:, :qw]
                            for hs in range(c.HPG)]
                    for kb, (pt, off) in enumerate(pts):
                        for hs in range(c.HPG):
                            nc.tensor.matmul(
                                psys[hs][:, off:],
                                v[:, kb, hp * c.HPG + hs, :],
                                pt[:, hs, off:],
                                start=(kb == 0),
                                stop=(kb == nkb - 1),
                            )
                    for hs in range(c.HPG):
                        h = hp * c.HPG + hs
                        yu = yup.tile([c.HD + 1, c.QT], F32, tag="yu",
                                      name=f"yu{hs}")[:, :qw]
                        nc.vector.tensor_copy(yu, psys[hs])
                        nc.gpsimd.dma_start(
                            l_all[h:h + 1, 0:qw], yu[c.HD:c.HD + 1, :])
                        yus[h] = yu

            def att_pass(q0, qw, hps, l_all, yus):
                """Fused S/exp/PV flow (one-group S^T lookahead) for the
                steady-state passes, whose QKV inputs are fully resident
                by the time they run."""
                nkb = (q0 + qw) // P
                for hp in hps:
                    psys = [ps_y.tile([c.HD + 1, c.QT], F32, tag="psy",
                                      name=f"psy{hs}")[:, :qw]
                            for hs in range(c.HPG)]

                    def s_pair(kb):
                        off = max(0, kb * P - q0)
                        pss = ps_s.tile([P, c.HPG, c.QT], F32, tag="pss",
                                        name="pss")[:, :, :qw]
                        for hs in range(c.HPG):
                            pb = hs * c.HD
                            nc.tensor.matmul(
                                pss[:, hs, off:],
                                kT[pb:pb + c.HD, hp, ts(kb, P)],
                                qT[pb:pb + c.HD, hp, ds(q0 + off,
                                                        qw - off)],
                                start=True, stop=True,
                            )
                        return pss, off

                    cur = s_pair(0)
                    for kb in range(nkb):
                        nxt = s_pair(kb + 1) if kb + 1 < nkb else None
                        pss, off = cur
                        pt = ptp.tile([P, c.HPG, c.QT], BF16, tag="pt",
                                      name="pt")[:, :, :qw]
                        nc.scalar.activation(
                            pt[:, :, off:], pss[:, :, off:],
                            AF.Exp, scale=c.scale)
                        if kb * P >= q0:
                            for hs in range(c.HPG):
                                nc.vector.tensor_tensor(
                                    pt[:, hs, off:off + P],
                                    pt[:, hs, off:off + P],
                                    mask_tri, ALU.mult,
                                )
                        for hs in range(c.HPG):
                            nc.tensor.matmul(
                                psys[hs][:, off:],
                                v[:, kb, hp * c.HPG + hs, :],
                                pt[:, hs, off:],
                                start=(kb == 0),
                                stop=(kb == nkb - 1),
                            )
                        cur = nxt
                    for hs in range(c.HPG):
                        h = hp * c.HPG + hs
                        yu = yup.tile([c.HD + 1, c.QT], F32, tag="yu",
                                      name=f"yu{hs}")[:, :qw]
                        nc.vector.tensor_copy(yu, psys[hs])
                        nc.gpsimd.dma_start(
                            l_all[h:h + 1, 0:qw], yu[c.HD:c.HD + 1, :])
                        yus[h] = yu

            def norm_pass(qw, yus, l_all):
                r_all = lrp.tile([c.H_LOC, c.QT], BF16, tag="rall",
                                 name="r_all")
                with nc.allow_low_precision(
                    reason="1/l rounded to bf16 for the broadcast matmul"
                ):
                    nc.vector.reciprocal(r_all[:, 0:qw], l_all[:, 0:qw])
                yt_q = ytp.tile([P, c.NHP, c.QT], BF16,
                                tag="yt_q", name="yt_q")[:, :, :qw]
                for hp in range(c.NHP):
                    psr = ps_mm.tile([P, max(c.TT, c.DH)], F32,
                                     tag="mm", name="psr")[:, :qw]
                    nc.tensor.matmul(
                        psr, esel_sb[:, ts(hp, P)], r_all[:, 0:qw],
                        start=True, stop=True,
                    )
                    for hs in range(c.HPG):
                        pb = hs * c.HD
                        nc.vector.tensor_tensor(
                            yt_q[pb:pb + c.HD, hp, :],
                            yus[hp * c.HPG + hs][0:c.HD, :],
                            psr[pb:pb + c.HD, :],
                            ALU.mult,
                        )
                return yt_q

            def ship_ag(qw, yt_q):
                # ship y^T to DRAM + pairwise AllGather along the dims
                # axis (runs on the CC cores, overlapped with compute)
                y_loc = drp.tile([c.DH, qw], BF16,
                                 tag=f"yloc{qw}", name="y_loc")
                nc.sync.dma_start(
                    y_loc.rearrange("(hp p) t -> p hp t", p=P), yt_q)
                if use_cc:
                    y_ag = drp.tile([c.GDH, qw], BF16,
                                    tag=f"ygat{qw}", name="y_ag")
                    nc.gpsimd.collective_compute(
                        "AllGather", ALU.bypass,
                        replica_groups=groups,
                        ins=[y_loc.opt()], outs=[y_ag.opt()],
                    )
                else:
                    y_ag = y_loc
                return y_ag

            def proj_pass(q0, qw, y_ag):
                # column-sharded proj on the gathered y^T; emitted one
                # pass later than its AllGather so a late AllGather can
                # never block the PE queue (the scheduler's cost model
                # underestimates collective latency).
                yag_sb = yagp.tile([P, c.GDH // P, c.QT], BF16,
                                   name="yag_sb")[:, :, :qw]
                nc.sync.dma_start(
                    yag_sb, y_ag.rearrange("(ch p) t -> p ch t", p=P))
                for tb in range(qw // P):
                    pso = ps_mm.tile([P, max(c.TT, c.DH)], F32,
                                     tag="mm", name="pso")[:, :c.DH]
                    for c2 in range(c.GDH // P):
                        nc.tensor.matmul(
                            pso,
                            yag_sb[:, c2, ts(tb, P)],
                            wp_sb[:, c2, :],
                            start=(c2 == 0),
                            stop=(not with_bias
                                  and c2 == c.GDH // P - 1),
                        )
                    if with_bias:
                        nc.tensor.matmul(
                            pso, ones_row[0:1, 0:P], bp_row,
                            start=False, stop=True,
                        )
                    osb = osbp.tile([P, c.DH], F32)
                    nc.vector.tensor_copy(osb, pso)
                    nc.sync.dma_start(
                        out[ds(q0 + tb * P, P), :], osb)

            # ===== the pipeline: the last tile is split in two query
            # halves so its AllGather+proj tail is halved; each tile's
            # QKV is emitted between the next attention pass and its
            # normalization so the PE has unblocked matmuls while the
            # AllGather flies and the reciprocal runs.
            passes = []
            for tt in range(c.NTT):
                if tt == c.NTT - 1 and c.QT >= 2 * P:
                    hqw = c.QT // 2
                    passes.append((tt, tt * c.QT, hqw, True))
                    passes.append((tt, tt * c.QT + hqw, hqw, False))
                else:
                    passes.append((tt, tt * c.QT, c.QT, True))

            pending_proj = None
            for tt, q0, qw, first in passes:
                if first:
                    prefetch_xt(tt + 1)
                l_all = lrp.tile([c.H_LOC, c.QT], F32, tag="lall",
                                 name="l_all")
                yus = {}
                if tt == 0 and first:
                    # pipeline head: each K/Q chain pair immediately
                    # unblocks its partition group's S^T + exp stream, so
                    # the ScalarE exp stream starts after just two chains.
                    # Group 0's PVs are deferred past the V chains (which
                    # the PE fills in while group 0's exps run).
                    for hp in range(c.NHP):
                        emit_kq(0, hp)
                        se = att_s_exp(q0, qw, [hp])
                        if hp == 0:
                            emit_v(0)
                        att_pv(q0, qw, se, l_all, yus)
                else:
                    att_pass(q0, qw, list(range(c.NHP)), l_all, yus)
                if first and tt + 1 < c.NTT:
                    for cb in range(c.CB):
                        emit_kq(tt + 1, cb)
                    emit_v(tt + 1)
                yt_q = norm_pass(qw, yus, l_all)
                y_ag = ship_ag(qw, yt_q)
                if pending_proj is not None:
                    proj_pass(*pending_proj)
                pending_proj = (q0, qw, y_ag)
            proj_pass(*pending_proj)

    nc.compile()
    return nc


def shard_inputs(c: Cfg, x, w_qkv, b_qkv, w_proj, b_proj, n_cores=8):
    """Full fp32 inputs -> per-core input maps (host-side marshalling).

    Matmul operands are cast to bf16 on the host; q/k biases stay fp32
    (applied on the f32 PSUM before the bf16 round)."""
    D, DH = c.D, c.DH
    oc = max(128, (c.T // 128) * c.H_LOC)
    ones = np.ones((128, oc), BF16NP)
    esel = np.zeros((c.H_LOC, c.NHP * 128), BF16NP)
    for h in range(c.H_LOC):
        hp, sub = h // c.HPG, h % c.HPG
        esel[h, hp * 128 + sub * c.HD: hp * 128 + (sub + 1) * c.HD] = 1

    maps = []
    for core in range(n_cores):
        b, hh = core // c.n_groups, core % c.n_groups
        sl = slice(hh * DH, (hh + 1) * DH)
        maps.append({
            "xT": np.ascontiguousarray(x[b].T).astype(BF16NP),
            "wq": np.ascontiguousarray(
                w_qkv[:, 0 * D:1 * D][:, sl]).astype(BF16NP),
            "wk": np.ascontiguousarray(
                w_qkv[:, 1 * D:2 * D][:, sl]).astype(BF16NP),
            "wv": np.ascontiguousarray(
                w_qkv[:, 2 * D:3 * D][:, sl]).astype(BF16NP),
            "bq": np.ascontiguousarray(
                b_qkv[0 * D:1 * D][sl], dtype=np.float32),
            "bk": np.ascontiguousarray(
                b_qkv[1 * D:2 * D][sl], dtype=np.float32),
            "bv": np.ascontiguousarray(
                b_qkv[2 * D:3 * D][sl]).reshape(1, DH).astype(BF16NP),
            "wp": np.ascontiguousarray(w_proj[:, sl]).astype(BF16NP),
            "bp": np.ascontiguousarray(
                b_proj[sl]).reshape(1, DH).astype(BF16NP),
            "onesin": ones,
            "esel": esel,
        })
    return maps


def gather_outputs(c: Cfg, results, n_cores=8):
    B = n_cores // c.n_groups
    out = np.empty((B, c.T, c.GDH), dtype=np.float32)
    for core in range(n_cores):
        b, hh = core // c.n_groups, core % c.n_groups
        out[b][:, hh * c.DH:(hh + 1) * c.DH] = results[core]["out"]
    return out


_NC_CACHE: dict = {}


def kernel(**inputs) -> np.ndarray:
    from concourse.bass_utils import run_bass_kernel_spmd

    c = FULL
    n_cores = 8
    wb = bool(np.any(inputs["b_qkv"]) or np.any(inputs["b_proj"]))
    key = (c, n_cores, wb)
    if key not in _NC_CACHE:
        _NC_CACHE[key] = build_nc(c, n_cores, with_bias=wb)
    nc = _NC_CACHE[key]
    in_maps = shard_inputs(
        c, inputs["x"], inputs["w_qkv"], inputs["b_qkv"],
        inputs["w_proj"], inputs["b_proj"], n_cores,
    )
    res = run_bass_kernel_spmd(
        nc, in_maps, core_ids=list(range(n_cores)),
        trace=bool(int(os.environ.get("KERNEL_TRACE", "0"))),
    )
    kernel.last_results = res
    return gather_outputs(c, res.results, n_cores)
